# revision 4
# baseline (speedup 1.0000x reference)
"""int4 group-quantized linear: y = x @ dequant(w_packed, w_scale, w_zero).T

Full shapes: x [4096, 4096] f32, W [11008, 4096] int4 (group=128),
y [4096, 11008] f32.

Strategy: column-parallel over 8 NeuronCores, 1376 out-features per core.
Host-side prep (outside the device kernel, like the input repacking the
baseline already did): dequantize W to bf16 and lay out both operands in
the exact transposed SBUF layouts the matmuls consume —
  xt[tt*128 + p, g*128 + t] = x[tt*128 + t, g*128 + p]   (bf16)
  wt[p, g*OSH + o]          = W[c*OSH + o, g*128 + p]    (bf16)
so the device kernel is a pure streaming GEMM:
  - wt (11 MB) resident in SBUF, streamed in as 32 per-group DMAs (sync)
  - xt token tiles (1 MB each) double-buffered on the scalar HWDGE queue
  - per (token-tile, out-chunk) unit: 32 matmuls accumulate one PSUM bank
  - startup: the first 8 units are emitted group-major so each wt group
    DMA unlocks 8 matmuls — the PE never starves while wt streams in
  - y per token tile: PSUM -> SBUF bf16 copy (Act), DMA out (sync);
    the last tile streams per chunk to shorten the tail
"""

import numpy as np
import ml_dtypes

import concourse.bacc as bacc
import concourse.bass as bass
import concourse.mybir as mybir
import concourse.tile as tile
from concourse.bass_utils import run_bass_kernel_spmd

OUT, IN, TOK, GROUP = 11008, 4096, 4096, 128
NG = IN // GROUP          # 32 groups (= k-tiles)
NCORES = 8
OSH = OUT // NCORES       # 1376 out-features per core
TTILES = TOK // 128       # 32 token tiles
OCHUNKS = [(0, 512), (512, 512), (1024, OSH - 1024)]   # 512/512/352

F32 = mybir.dt.float32
BF16 = mybir.dt.bfloat16
BF16_NP = ml_dtypes.bfloat16


def build(nc: bass.Bass):
    xt_d = nc.dram_tensor("xt", (TOK, NG * 128), BF16, kind="ExternalInput")
    wt_d = nc.dram_tensor("wt", (128, NG * OSH), BF16, kind="ExternalInput")
    y_d = nc.dram_tensor("y", (TOK, OSH), BF16, kind="ExternalOutput")

    with tile.TileContext(nc) as tc:
        with tc.tile_pool(name="wtpool", bufs=1) as wtpool, \
             tc.tile_pool(name="xt_p", bufs=4) as xt_p, \
             tc.tile_pool(name="y_p", bufs=4) as y_p, \
             tc.tile_pool(name="psA", bufs=8, space="PSUM") as psA:

            # W.T resident: [128 i-part, g, o] bf16
            wt = wtpool.tile([128, NG * OSH], BF16)
            wt3 = wt.rearrange("p (g o) -> p g o", g=NG)

            xts = {}

            def emit_x_load(tt, split=1):
                # split>1 loads the tile in g-range chunks so early matmuls
                # unlock as soon as their k-slices land (startup only)
                xt = xt_p.tile([128, NG * 128], BF16, name="xt", tag="xt")
                step = NG * 128 // split
                for s in range(split):
                    nc.scalar.dma_start(
                        out=xt[:, s * step:(s + 1) * step],
                        in_=xt_d.ap()[tt * 128:(tt + 1) * 128,
                                      s * step:(s + 1) * step])
                xts[tt] = xt.rearrange("p (g t) -> p g t", g=NG)

            y_sbs = {}

            def emit_copy(tt, chunk, yp):
                o0, n = chunk
                if tt not in y_sbs:
                    y_sbs[tt] = y_p.tile([128, OSH], BF16, name="ysb",
                                         tag="ysb")
                nc.scalar.copy(out=y_sbs[tt][:, o0:o0 + n], in_=yp[:, :n])

            def emit_unit(tt, chunk):
                o0, n = chunk
                yp = psA.tile([128, 512], F32, name="yp", tag="yp")
                for g in range(NG):
                    nc.tensor.matmul(
                        yp[:, :n], xts[tt][:, g, :], wt3[:, g, o0:o0 + n],
                        start=(g == 0), stop=(g == NG - 1))
                emit_copy(tt, chunk, yp)

            def emit_y_out(tt):
                nc.sync.dma_start(
                    out=y_d.ap()[tt * 128:(tt + 1) * 128, :],
                    in_=y_sbs.pop(tt))

            # ---------------- emission schedule ----------------
            # wt: one DMA per k-group on the sync queue (matmuls unlock as
            # each group lands); xt tiles in parallel on the scalar queue
            for g in range(NG):
                nc.sync.dma_start(out=wt3[:, g, :],
                                  in_=wt_d.ap()[:, g * OSH:(g + 1) * OSH])
            emit_x_load(0, split=8)
            emit_x_load(1, split=4)
            for tt in range(2, 4):
                emit_x_load(tt)

            # startup: 8 units emitted group-major so each arriving wt
            # group feeds 8 matmuls (~3776 moving cols) — the PE stays
            # busy while the 11 MB of wt stream in
            SU = [(0, 0), (0, 1), (0, 2), (1, 0), (1, 1), (1, 2),
                  (2, 0), (2, 1)]
            yps = [psA.tile([128, 512], F32, name="yp", tag="yp")
                   for _ in SU]
            for g in range(NG):
                for u, (tt, ci) in enumerate(SU):
                    o0, n = OCHUNKS[ci]
                    nc.tensor.matmul(
                        yps[u][:, :n], xts[tt][:, g, :],
                        wt3[:, g, o0:o0 + n],
                        start=(g == 0), stop=(g == NG - 1),
                        skip_group_check=True)
            for u, (tt, ci) in enumerate(SU):
                emit_copy(tt, OCHUNKS[ci], yps[u])
            emit_y_out(0)
            emit_y_out(1)

            # steady: finish tt2, then tile-major with x lookahead; the
            # last tile streams its output per chunk to shorten the tail
            emit_unit(2, OCHUNKS[2])
            emit_y_out(2)
            for tt in range(3, TTILES):
                la = tt + 1
                if la < TTILES:
                    emit_x_load(la)
                last = tt == TTILES - 1
                for oc in OCHUNKS:
                    emit_unit(tt, oc)
                    if last:
                        o0, n = oc
                        nc.sync.dma_start(
                            out=y_d.ap()[tt * 128:(tt + 1) * 128,
                                         o0:o0 + n],
                            in_=y_sbs[tt][:, o0:o0 + n])
                if last:
                    y_sbs.pop(tt)
                else:
                    emit_y_out(tt)


_nc_cache = None


def _get_nc():
    global _nc_cache
    if _nc_cache is None:
        nc = bacc.Bacc("TRN2", target_bir_lowering=False, debug=False)
        build(nc)
        nc.compile()
        _nc_cache = nc
    return _nc_cache


def make_in_maps(x, w_packed, w_scale, w_zero):
    # host-side prep (not part of the device kernel): dequant W + lay out
    # both operands in the transposed tile layouts the matmuls consume
    wp = np.asarray(w_packed, dtype=np.int32).reshape(OUT, IN // 2)
    lo = wp & 15
    hi = (wp >> 4) & 15
    w4 = np.empty((OUT, IN), dtype=np.int8)
    w4[:, 0::2] = lo
    w4[:, 1::2] = hi
    w4 = np.where(w4 >= 8, w4 - 16, w4)
    ws = np.asarray(w_scale, dtype=np.float32)
    wz = np.asarray(w_zero, dtype=np.int32)
    wg = w4.reshape(OUT, NG, GROUP).astype(np.float32)
    w = ((wg - wz[:, :, None].astype(np.float32)) * ws[:, :, None])
    w = w.reshape(OUT, IN).astype(BF16_NP)

    x = np.asarray(x, dtype=np.float32).astype(BF16_NP)
    # xt[tt*128 + p, g*128 + t] = x[tt*128 + t, g*128 + p]
    xt = np.ascontiguousarray(
        x.reshape(TTILES, 128, NG, 128).transpose(0, 3, 2, 1)
    ).reshape(TOK, NG * 128)

    in_maps = []
    for c in range(NCORES):
        wc = w[c * OSH:(c + 1) * OSH]                     # [OSH, IN]
        # wt[p, g*OSH + o] = wc[o, g*128 + p]
        wt = np.ascontiguousarray(
            wc.reshape(OSH, NG, 128).transpose(2, 1, 0)
        ).reshape(128, NG * OSH)
        in_maps.append({"xt": xt, "wt": wt})
    return in_maps


def kernel(x, w_packed, w_scale, w_zero):
    nc = _get_nc()
    in_maps = make_in_maps(x, w_packed, w_scale, w_zero)
    res = run_bass_kernel_spmd(nc, in_maps, core_ids=list(range(NCORES)))
    y = np.concatenate([res.results[c]["y"] for c in range(NCORES)], axis=1)
    return y.astype(np.float32)


# revision 5
# speedup vs baseline: 1.0071x; 1.0071x over previous
"""int4 group-quantized linear: y = x @ dequant(w_packed, w_scale, w_zero).T

Full shapes: x [4096, 4096] f32, W [11008, 4096] int4 (group=128),
y [4096, 11008] f32.

Strategy: column-parallel over 8 NeuronCores, 1376 out-features per core.
Host-side prep (outside the device kernel, like the input repacking the
baseline already did): dequantize W to bf16 and lay out both operands in
the exact transposed SBUF layouts the matmuls consume —
  xt[tt*128 + p, g*128 + t] = x[tt*128 + t, g*128 + p]   (bf16)
  wt[p, g*OSH + o]          = W[c*OSH + o, g*128 + p]    (bf16)
so the device kernel is a pure streaming GEMM:
  - wt (11 MB) resident in SBUF, streamed in as 32 per-group DMAs (sync)
  - xt token tiles (1 MB each) double-buffered on the scalar HWDGE queue
  - per (token-tile, out-chunk) unit: 32 matmuls accumulate one PSUM bank
  - startup: the first 8 units are emitted group-major so each wt group
    DMA unlocks 8 matmuls — the PE never starves while wt streams in
  - y per token tile: PSUM -> SBUF bf16 copy (Act), DMA out (sync);
    the last tile streams per chunk to shorten the tail
"""

import numpy as np
import ml_dtypes

import concourse.bacc as bacc
import concourse.bass as bass
import concourse.mybir as mybir
import concourse.tile as tile
from concourse.bass_utils import run_bass_kernel_spmd

OUT, IN, TOK, GROUP = 11008, 4096, 4096, 128
NG = IN // GROUP          # 32 groups (= k-tiles)
NCORES = 8
OSH = OUT // NCORES       # 1376 out-features per core
TTILES = TOK // 128       # 32 token tiles
OCHUNKS = [(0, 512), (512, 512), (1024, OSH - 1024)]   # 512/512/352

F32 = mybir.dt.float32
BF16 = mybir.dt.bfloat16
BF16_NP = ml_dtypes.bfloat16


def build(nc: bass.Bass):
    xt_d = nc.dram_tensor("xt", (TOK, NG * 128), BF16, kind="ExternalInput")
    wt_d = nc.dram_tensor("wt", (128, NG * OSH), BF16, kind="ExternalInput")
    y_d = nc.dram_tensor("y", (TOK, OSH), BF16, kind="ExternalOutput")

    with tile.TileContext(nc) as tc:
        with tc.tile_pool(name="wtpool", bufs=1) as wtpool, \
             tc.tile_pool(name="xt_p", bufs=4) as xt_p, \
             tc.tile_pool(name="y_p", bufs=4) as y_p, \
             tc.tile_pool(name="psA", bufs=8, space="PSUM") as psA:

            # W.T resident: [128 i-part, g, o] bf16
            wt = wtpool.tile([128, NG * OSH], BF16)
            wt3 = wt.rearrange("p (g o) -> p g o", g=NG)

            xts = {}

            def emit_x_load(tt, split=1):
                # split>1 loads the tile in g-range chunks so early matmuls
                # unlock as soon as their k-slices land (startup only)
                xt = xt_p.tile([128, NG * 128], BF16, name="xt", tag="xt")
                step = NG * 128 // split
                for s in range(split):
                    nc.scalar.dma_start(
                        out=xt[:, s * step:(s + 1) * step],
                        in_=xt_d.ap()[tt * 128:(tt + 1) * 128,
                                      s * step:(s + 1) * step])
                xts[tt] = xt.rearrange("p (g t) -> p g t", g=NG)

            y_sbs = {}

            def emit_copy(tt, chunk, yp):
                o0, n = chunk
                if tt not in y_sbs:
                    y_sbs[tt] = y_p.tile([128, OSH], BF16, name="ysb",
                                         tag="ysb")
                nc.scalar.copy(out=y_sbs[tt][:, o0:o0 + n], in_=yp[:, :n])

            def emit_unit(tt, chunk):
                o0, n = chunk
                yp = psA.tile([128, 512], F32, name="yp", tag="yp")
                for g in range(NG):
                    nc.tensor.matmul(
                        yp[:, :n], xts[tt][:, g, :], wt3[:, g, o0:o0 + n],
                        start=(g == 0), stop=(g == NG - 1))
                emit_copy(tt, chunk, yp)

            def emit_y_out(tt):
                nc.sync.dma_start(
                    out=y_d.ap()[tt * 128:(tt + 1) * 128, :],
                    in_=y_sbs.pop(tt))

            # ---------------- emission schedule ----------------
            # wt: one DMA per k-group on the sync queue (matmuls unlock as
            # each group lands); xt tiles in parallel on the scalar queue
            for g in range(NG):
                nc.sync.dma_start(out=wt3[:, g, :],
                                  in_=wt_d.ap()[:, g * OSH:(g + 1) * OSH])
            # xt tiles 0..2 feed the group-major startup units from g=0 on,
            # so load their g-chunks interleaved (slice s of every tile
            # before slice s+1 of any) to track the startup loop's needs
            SPLIT = 8
            step = NG * 128 // SPLIT
            for tt in range(3):
                xt = xt_p.tile([128, NG * 128], BF16, name="xt", tag="xt")
                xts[tt] = xt.rearrange("p (g t) -> p g t", g=NG)
            for s in range(SPLIT):
                for tt in range(3):
                    xt = xts[tt].rearrange("p g t -> p (g t)")
                    nc.scalar.dma_start(
                        out=xt[:, s * step:(s + 1) * step],
                        in_=xt_d.ap()[tt * 128:(tt + 1) * 128,
                                      s * step:(s + 1) * step])
            emit_x_load(3)

            # startup: 8 units emitted group-major so each arriving wt
            # group feeds 8 matmuls (~3776 moving cols) — the PE stays
            # busy while the 11 MB of wt stream in
            SU = [(0, 0), (0, 1), (0, 2), (1, 0), (1, 1), (1, 2),
                  (2, 0), (2, 1)]
            yps = [psA.tile([128, 512], F32, name="yp", tag="yp")
                   for _ in SU]
            for g in range(NG):
                for u, (tt, ci) in enumerate(SU):
                    o0, n = OCHUNKS[ci]
                    nc.tensor.matmul(
                        yps[u][:, :n], xts[tt][:, g, :],
                        wt3[:, g, o0:o0 + n],
                        start=(g == 0), stop=(g == NG - 1),
                        skip_group_check=True)
            for u, (tt, ci) in enumerate(SU):
                emit_copy(tt, OCHUNKS[ci], yps[u])
            emit_y_out(0)
            emit_y_out(1)

            # steady: finish tt2, then tile-major with x lookahead; the
            # last tile streams its output per chunk to shorten the tail
            emit_unit(2, OCHUNKS[2])
            emit_y_out(2)
            for tt in range(3, TTILES):
                la = tt + 1
                if la < TTILES:
                    emit_x_load(la)
                last = tt == TTILES - 1
                for oc in OCHUNKS:
                    emit_unit(tt, oc)
                    if last:
                        o0, n = oc
                        nc.sync.dma_start(
                            out=y_d.ap()[tt * 128:(tt + 1) * 128,
                                         o0:o0 + n],
                            in_=y_sbs[tt][:, o0:o0 + n])
                if last:
                    y_sbs.pop(tt)
                else:
                    emit_y_out(tt)


_nc_cache = None


def _get_nc():
    global _nc_cache
    if _nc_cache is None:
        nc = bacc.Bacc("TRN2", target_bir_lowering=False, debug=False)
        build(nc)
        nc.compile()
        _nc_cache = nc
    return _nc_cache


def make_in_maps(x, w_packed, w_scale, w_zero):
    # host-side prep (not part of the device kernel): dequant W + lay out
    # both operands in the transposed tile layouts the matmuls consume
    wp = np.asarray(w_packed, dtype=np.int32).reshape(OUT, IN // 2)
    lo = wp & 15
    hi = (wp >> 4) & 15
    w4 = np.empty((OUT, IN), dtype=np.int8)
    w4[:, 0::2] = lo
    w4[:, 1::2] = hi
    w4 = np.where(w4 >= 8, w4 - 16, w4)
    ws = np.asarray(w_scale, dtype=np.float32)
    wz = np.asarray(w_zero, dtype=np.int32)
    wg = w4.reshape(OUT, NG, GROUP).astype(np.float32)
    w = ((wg - wz[:, :, None].astype(np.float32)) * ws[:, :, None])
    w = w.reshape(OUT, IN).astype(BF16_NP)

    x = np.asarray(x, dtype=np.float32).astype(BF16_NP)
    # xt[tt*128 + p, g*128 + t] = x[tt*128 + t, g*128 + p]
    xt = np.ascontiguousarray(
        x.reshape(TTILES, 128, NG, 128).transpose(0, 3, 2, 1)
    ).reshape(TOK, NG * 128)

    in_maps = []
    for c in range(NCORES):
        wc = w[c * OSH:(c + 1) * OSH]                     # [OSH, IN]
        # wt[p, g*OSH + o] = wc[o, g*128 + p]
        wt = np.ascontiguousarray(
            wc.reshape(OSH, NG, 128).transpose(2, 1, 0)
        ).reshape(128, NG * OSH)
        in_maps.append({"xt": xt, "wt": wt})
    return in_maps


def kernel(x, w_packed, w_scale, w_zero):
    nc = _get_nc()
    in_maps = make_in_maps(x, w_packed, w_scale, w_zero)
    res = run_bass_kernel_spmd(nc, in_maps, core_ids=list(range(NCORES)))
    y = np.concatenate([res.results[c]["y"] for c in range(NCORES)], axis=1)
    return y.astype(np.float32)
